# revision 16
# baseline (speedup 1.0000x reference)
"""Multi-head causal attention on 8 Trainium2 NeuronCores.

Problem: B=2, T=2048, C=1024, H=16, HS=64 (fp32), causal mask.

Sharding: 8 cores = 2 batches x 4 head-groups (4 heads each). Each core
computes q/k/v projections + attention + its partial output projection for
its 4 heads of its batch; the host sums the 4 per-batch partials (the
all-reduce of the tensor-parallel output projection) and adds the bias.

Per-core kernel dataflow (everything "transposed", T on the free axis):
  qT/kT [heads(64)x2, T] = W.T @ xT          (PE, K=C chunks of 128)
  v     [T, h, 64+one]                       (PE + strided DVE copy)
  scores: per ts-chunk, BOTH heads of a pair as two K=64 row-tiled
    matmuls (rows 0-63 / 64-127 run concurrently in the PE array);
    causal chunks narrowed to the valid tq range   -> exp (ACT)
  causal: affine_select only on the 128x128 diagonal sub-block (GPSIMD)
  attnT_aug [65, tq] += v_aug.T @ expT       (PE, ones column => row 64 = denom)
  recip = 1/denom straight from PSUM row 64 (DVE), broadcast over 64
    partitions via a K=1 matmul (PE), attnT = attn * recip (DVE)
  y_partial [tq, C] = attnT_pair.T @ wproj   (PE, K=128 per head-pair)

float32r = full-precision fp32 matmul at 1 cycle/row (vs 4 for plain fp32).
"""

import numpy as np

B, T, C, H, HS = 2, 2048, 1024, 16, 64
NCORES = 8
HPC = 4            # heads per core
NKC = C // 128     # contraction chunks (8)
NJ = T // 512      # tq chunks (4)
NTS = T // 128     # ts chunks (16)

_NC_CACHE = {}


def _build_nc():
    if "nc" in _NC_CACHE:
        return _NC_CACHE["nc"]
    from contextlib import ExitStack
    import concourse.bass as bass
    from concourse import bacc, tile, mybir

    f32 = mybir.dt.float32
    f32r = mybir.dt.float32r
    EXP = mybir.ActivationFunctionType.Exp

    nc = bacc.Bacc("TRN2", target_bir_lowering=False, debug=False,
                   enable_asserts=False, num_devices=NCORES)

    xT_d = nc.dram_tensor("xT", (C, T), f32, kind="ExternalInput").ap()
    wq_d = nc.dram_tensor("wq_s", (C, HPC * HS), f32, kind="ExternalInput").ap()
    wk_d = nc.dram_tensor("wk_s", (C, HPC * HS), f32, kind="ExternalInput").ap()
    wv_d = nc.dram_tensor("wv_s", (C, HPC * HS), f32, kind="ExternalInput").ap()
    wp_d = nc.dram_tensor("wp_s", (HPC * HS, C), f32, kind="ExternalInput").ap()
    y_d = nc.dram_tensor("y", (T, C), f32, kind="ExternalOutput").ap()

    scale = float(1.0 / np.sqrt(HS))

    with tile.TileContext(nc) as tc, ExitStack() as ctx:
        persist = ctx.enter_context(tc.tile_pool(name="persist", bufs=1))
        work = ctx.enter_context(tc.tile_pool(name="work", bufs=3))
        small = ctx.enter_context(tc.tile_pool(name="small", bufs=2))
        outp = ctx.enter_context(tc.tile_pool(name="outp", bufs=2))
        spap = ctx.enter_context(tc.tile_pool(name="spap", bufs=3))
        psp = ctx.enter_context(tc.tile_pool(name="psp", bufs=2, space="PSUM"))
        psaux = ctx.enter_context(tc.tile_pool(name="psaux", bufs=2, space="PSUM"))
        psatt = ctx.enter_context(tc.tile_pool(name="psatt", bufs=2, space="PSUM"))

        # ---- persistent SBUF tensors (f32r = fast-fp32 PE path) ----
        xt = [persist.tile([128, T], f32r, tag=f"xt{c}", name=f"xt{c}") for c in range(NKC)]
        wq_sb = persist.tile([128, NKC, 256], f32r, tag="wq")
        wk_sb = persist.tile([128, NKC, 256], f32r, tag="wk")
        wv_sb = persist.tile([128, NKC, 256], f32r, tag="wv")
        wp_sb = persist.tile([128, 2, C], f32r, tag="wp")
        qT = [persist.tile([128, T], f32r, tag=f"qT{p}", name=f"qT{p}") for p in range(2)]
        kT = [persist.tile([128, T], f32r, tag=f"kT{p}", name=f"kT{p}") for p in range(2)]
        # v: [ts-chunk(16) x head(4) x (64 vals + ones col)] per 128 ts partitions
        vt = persist.tile([128, NTS, HPC, 65], f32r, tag="vt")
        attnT = [persist.tile([128, T], f32r, tag=f"attnT{p}", name=f"attnT{p}") for p in range(2)]
        ones64 = persist.tile([1, 64], f32r, tag="ones64")
        ones_f32 = persist.tile([128, 64], f32, tag="ones_f32")

        # ---- loads: striped across sync+scalar in consumption order; gpsimd
        # carries only late-needed wp so it can't steal HBM bandwidth from
        # the critical path (wq + x first halves) ----
        for eng, par in ((nc.sync, 0), (nc.scalar, 1)):
            for c in (par, par + 2):
                eng.dma_start(out=wq_sb[:, c, :],
                              in_=wq_d[c * 128:(c + 1) * 128, :].bitcast(f32r))
            eng.dma_start(out=xt[par][:, 0:1024],
                          in_=xT_d[par * 128:(par + 1) * 128, 0:1024].bitcast(f32r))
            for c in (par + 4, par + 6):
                eng.dma_start(out=wq_sb[:, c, :],
                              in_=wq_d[c * 128:(c + 1) * 128, :].bitcast(f32r))
            for c in (par + 2, par + 4, par + 6):
                eng.dma_start(out=xt[c][:, 0:1024],
                              in_=xT_d[c * 128:(c + 1) * 128, 0:1024].bitcast(f32r))
            for c in range(par, NKC, 2):
                eng.dma_start(out=wk_sb[:, c, :],
                              in_=wk_d[c * 128:(c + 1) * 128, :].bitcast(f32r))
            for c in range(par, NKC, 2):
                eng.dma_start(out=wv_sb[:, c, :],
                              in_=wv_d[c * 128:(c + 1) * 128, :].bitcast(f32r))
            for c in range(par, NKC, 2):
                eng.dma_start(out=xt[c][:, 1024:2048],
                              in_=xT_d[c * 128:(c + 1) * 128, 1024:2048].bitcast(f32r))
        nc.gpsimd.dma_start(out=wp_sb, in_=wp_d.rearrange("(k p) n -> p k n", p=128).bitcast(f32r))

        # f32r matmul operands must come from rounding producers (DVE copy),
        # so memset an f32 tile and cast-copy the ones into place
        nc.vector.memset(ones_f32, 1.0)
        nc.vector.tensor_copy(out=ones64, in_=ones_f32[0:1, 0:64])
        nc.vector.tensor_copy(
            out=vt[:, :, :, 64:65],
            in_=ones_f32.rearrange("p (t h o) -> p t h o", t=NTS, h=HPC, o=1))

        # ---- PE warm-up: dummy matmuls on zeros keep the HAM activity
        # monitor hot through the DMA-only head, so real matmuls start at
        # 2.4 GHz instead of paying the 1.2 GHz cold ramp ----
        warm_f = persist.tile([128, 512], f32, tag="warm_f")
        nc.vector.memset(warm_f, 0.0)
        warm_r = persist.tile([128, 512], f32r, tag="warm_r")
        nc.vector.tensor_copy(out=warm_r, in_=warm_f)
        warm_ps = psaux.tile([128, 512], f32, tag="aux", name="warm_ps")
        for _ in range(60):
            nc.tensor.matmul(warm_ps, lhsT=warm_r[:, 0:128], rhs=warm_r,
                             start=True, stop=True)

        # ---------- emission helpers ----------
        filler = []     # queue of closures emitting independent PE work

        def pull(n):
            for _ in range(n):
                if filler:
                    filler.pop(0)()

        def qk_chain_units(pair, dst, w_sb, J, name):
            # split one 8-matmul accumulation chain into 4 filler units
            ps = psaux.tile([128, 512], f32, tag="aux", name=name)

            def unit(c0):
                def f():
                    for c in (c0, c0 + 1):
                        nc.tensor.matmul(
                            ps,
                            lhsT=w_sb[:, c, 128 * pair:128 * pair + 128],
                            rhs=xt[c][:, 512 * J:512 * J + 512],
                            start=(c == 0), stop=(c == NKC - 1))
                    if c0 == NKC - 2:
                        nc.vector.tensor_copy(
                            out=dst[:, 512 * J:512 * J + 512], in_=ps)
                return f
            return [unit(c0) for c0 in range(0, NKC, 2)]

        def qk_chain(pair, dst, w_sb, J, name):
            for u in qk_chain_units(pair, dst, w_sb, J, name):
                u()

        def v_chain(t):
            ps = psaux.tile([128, 512], f32, tag="aux", name=f"v_{t}")
            for c in range(NKC):
                nc.tensor.matmul(
                    ps[:, 0:256],
                    lhsT=xt[c][:, 128 * t:128 * t + 128],
                    rhs=wv_sb[:, c, :],
                    start=(c == 0), stop=(c == NKC - 1))
            nc.vector.tensor_copy(
                out=vt[:, t, :, 0:64],
                in_=ps[:, 0:256].rearrange("p (h d) -> p h d", h=HPC))

        def proj_tile(m, n):
            py_ = psaux.tile([128, 512], f32, tag="aux", name=f"y_{m}_{n}")
            for pair in range(2):
                nc.tensor.matmul(
                    py_,
                    lhsT=attnT[pair][:, 128 * m:128 * m + 128],
                    rhs=wp_sb[:, pair, 512 * n:512 * n + 512],
                    start=(pair == 0), stop=(pair == 1))
            yo = outp.tile([128, 512], f32, tag="yo")
            nc.vector.tensor_copy(out=yo, in_=py_)
            nc.sync.dma_start(
                out=y_d[128 * m:128 * m + 128, 512 * n:512 * n + 512], in_=yo)

        def att_block(pair, J, extra=1):
            """Both heads of `pair` for tq chunk J: per ts-chunk, two K=64
            row-tiled score matmuls (concurrent in the PE array), one batched
            exp, diagonal-narrowed AV accumulation with a ones column giving
            the softmax denominator in row 64."""
            nch = 4 * J + 4
            pa = [psatt.tile([65, 512], f32, tag="att", name=f"pa{hh}_{pair}_{J}")
                  for hh in (0, 1)]

            def do_av(et, t, last):
                d = t - 4 * J
                off = 128 * d if d > 0 else 0
                w = 512 - off
                for hh in (0, 1):
                    base = off if hh == 0 else 512
                    nc.tensor.matmul(
                        pa[hh][:, off:512],
                        lhsT=vt[:, t, 2 * pair + hh, :],
                        rhs=et[:, base:base + w],
                        start=(t == 0), stop=last)

            pend = None          # (et, t) AV one step behind scores
            for t in range(nch):
                d = t - 4 * J
                off = 128 * d if d > 0 else 0
                w = 512 - off
                ss = psp.tile([128, 1024], f32, tag="s", name=f"ss_{pair}_{J}_{t}")
                for hh in (0, 1):
                    # hh0 at cols [off,512), hh1 packed right after at [512,512+w)
                    base = off if hh == 0 else 512
                    nc.tensor.matmul(
                        ss[:, base:base + w],
                        lhsT=kT[pair][64 * hh:64 * hh + 64, 128 * t:128 * t + 128],
                        rhs=qT[pair][64 * hh:64 * hh + 64, 512 * J + off:512 * J + 512],
                        start=True, stop=True)
                et = work.tile([128, 1024], f32r, tag="et", bufs=3)
                nc.scalar.activation(out=et[:, off:512 + w], in_=ss[:, off:512 + w],
                                     func=EXP, scale=scale)
                if d >= 0:
                    for hh in (0, 1):
                        base = off if hh == 0 else 512
                        sl = et[:, base:base + 128]
                        # within the diagonal 128x128 sub-block keep f >= p
                        nc.gpsimd.affine_select(
                            out=sl, in_=sl,
                            compare_op=mybir.AluOpType.is_ge,
                            fill=0.0, base=0,
                            pattern=[[1, 128]], channel_multiplier=-1)
                if pend is not None:
                    do_av(*pend, False)
                pend = (et, t)
                pull(extra)
            do_av(*pend, True)
            # softmax denominator: copy pa to SBUF (frees the PSUM bank for the
            # next block), recip row 64, broadcast via K=1 matmul, normalize
            rcr, spa = [], []
            for hh in (0, 1):
                sp = spap.tile([65, 512], f32, tag="spa", name=f"spa{hh}_{pair}_{J}")
                nc.vector.tensor_copy(out=sp, in_=pa[hh])
                spa.append(sp)
                # custom-DVE ops read the wrong partition if in/out bases
                # mismatch: plain-copy the denom row down to partition 0 first
                den = small.tile([1, 512], f32, tag="den", name=f"den{hh}_{pair}_{J}")
                nc.vector.tensor_copy(out=den, in_=sp[64:65, :])
                r = small.tile([1, 512], f32, tag="rc", name=f"rc{hh}_{pair}_{J}")
                nc.vector.reciprocal_approx_fast(out=r, in_=den)
                rr = small.tile([1, 512], f32r, tag="rcr", name=f"rcr{hh}_{pair}_{J}")
                nc.vector.tensor_copy(out=rr, in_=r)
                rcr.append(rr)
            pull(1)
            for hh in (0, 1):
                rps = psaux.tile([64, 512], f32, tag="aux", name=f"rps{hh}_{pair}_{J}")
                nc.tensor.matmul(rps, lhsT=ones64, rhs=rcr[hh],
                                 start=True, stop=True)
                if hh == 0:
                    nc.vector.tensor_mul(
                        attnT[pair][0:64, 512 * J:512 * J + 512], spa[0][0:64, :], rps)
                else:
                    tmp = small.tile([64, 512], f32r, tag="tmp")
                    nc.vector.tensor_mul(tmp, spa[1][0:64, :], rps)
                    nc.sync.dma_start(
                        out=attnT[pair][64:128, 512 * J:512 * J + 512], in_=tmp)
                pull(1)

        # ---------- phase A: pair0 q/k for the left half, v left chunks ----------
        qk_chain(0, qT[0], wq_sb, 0, "q0_0")
        qk_chain(0, qT[0], wq_sb, 1, "q0_1")
        qk_chain(0, kT[0], wk_sb, 0, "k0_0")
        qk_chain(0, kT[0], wk_sb, 1, "k0_1")
        for t in range(8):
            v_chain(t)

        # ---------- phase B: attention(pair0) ascending J ----------
        # fillers sized to B's act-latency deficit, in data-arrival order;
        # later-needed chains are emitted inline between blocks instead so
        # phase C's big first block isn't starved
        filler.extend(qk_chain_units(0, qT[0], wq_sb, 2, "q0_2"))
        filler.extend(qk_chain_units(0, kT[0], wk_sb, 2, "k0_2"))
        for t in range(8, 12):
            filler.append(lambda t=t: v_chain(t))
        filler.extend(qk_chain_units(0, qT[0], wq_sb, 3, "q0_3"))
        filler.extend(qk_chain_units(0, kT[0], wk_sb, 3, "k0_3"))
        for t in range(12, NTS):
            filler.append(lambda t=t: v_chain(t))
        filler.extend(qk_chain_units(1, kT[1], wk_sb, 0, "k1_0"))
        filler.extend(qk_chain_units(1, kT[1], wk_sb, 1, "k1_1"))
        att_block(0, 0, extra=2)
        att_block(0, 1, extra=1)
        att_block(0, 2, extra=1)
        att_block(0, 3, extra=1)

        # ---------- phase C: attention(pair1) descending J ----------
        # per-block prerequisites run inline (they fill the previous block's
        # activation drain); proj tiles for finished J rows become fillers
        qk_chain(1, kT[1], wk_sb, 2, "k1_2")
        qk_chain(1, kT[1], wk_sb, 3, "k1_3")
        qk_chain(1, qT[1], wq_sb, 3, "q1_3")
        att_block(1, 3)
        filler.extend(
            (lambda m=m, n=n: (lambda: proj_tile(m, n)))()
            for m in range(12, 16) for n in range(2))
        qk_chain(1, qT[1], wq_sb, 2, "q1_2")
        att_block(1, 2)
        filler.extend(
            (lambda m=m, n=n: (lambda: proj_tile(m, n)))()
            for m in range(8, 12) for n in range(2))
        qk_chain(1, qT[1], wq_sb, 1, "q1_1")
        att_block(1, 1)
        filler.extend(
            (lambda m=m, n=n: (lambda: proj_tile(m, n)))()
            for m in range(4, 8) for n in range(2))
        qk_chain(1, qT[1], wq_sb, 0, "q1_0")
        att_block(1, 0)
        filler.extend(
            (lambda m=m, n=n: (lambda: proj_tile(m, n)))()
            for m in range(0, 4) for n in range(2))
        pull(len(filler))

    nc.compile()
    _NC_CACHE["nc"] = nc
    return nc


def make_in_maps(x, wq, wk, wv, wproj):
    xTs = [np.ascontiguousarray(x[b].T) for b in range(B)]
    in_maps = []
    for core in range(NCORES):
        b, g = divmod(core, 4)
        hs = slice(4 * g, 4 * g + 4)
        in_maps.append({
            "xT": xTs[b],
            "wq_s": np.ascontiguousarray(wq[hs].transpose(1, 0, 2).reshape(C, HPC * HS)),
            "wk_s": np.ascontiguousarray(wk[hs].transpose(1, 0, 2).reshape(C, HPC * HS)),
            "wv_s": np.ascontiguousarray(wv[hs].transpose(1, 0, 2).reshape(C, HPC * HS)),
            "wp_s": np.ascontiguousarray(wproj[4 * g * HS:(4 * g + 4) * HS, :]),
        })
    return in_maps


def _assemble(results, bproj):
    y = np.zeros((B, T, C), dtype=np.float32)
    for core in range(NCORES):
        y[core // 4] += results[core]["y"]
    y += bproj.astype(np.float32)[None, None, :]
    return y


def _is_causal(attention_mask):
    tril = np.tril(np.ones((T, T), dtype=bool))
    return all(np.array_equal(attention_mask[b], tril) for b in range(B))


def _numpy_fallback(x, attention_mask, wq, wk, wv, wproj, bproj):
    x64 = x.astype(np.float32)
    q = np.einsum('btc,hcd->bhtd', x64, wq)
    k = np.einsum('btc,hcd->bhtd', x64, wk)
    v = np.einsum('btc,hcd->bhtd', x64, wv)
    wei = np.einsum('bhtd,bhsd->bhts', q, k) / np.sqrt(np.float32(HS))
    wei = np.where(attention_mask[:, None, :, :], wei, -np.inf)
    wei = wei - wei.max(axis=-1, keepdims=True)
    wei = np.exp(wei)
    wei = wei / wei.sum(axis=-1, keepdims=True)
    out = np.einsum('bhts,bhsd->bhtd', wei, v)
    out = out.transpose(0, 2, 1, 3).reshape(B, T, H * HS)
    return (out @ wproj + bproj).astype(np.float32)


def _install_ntff_hook():
    """Recreate the antenv.axon_hooks shim so trace=True works under axon."""
    import sys, types
    try:
        from antenv.axon_hooks import get_axon_ntff_profile_hook  # noqa
        return
    except ImportError:
        pass
    import antenv
    mod = types.ModuleType("antenv.axon_hooks")
    holder = [None]
    mod.set_axon_ntff_profile_hook = lambda h: holder.__setitem__(0, h)
    mod.get_axon_ntff_profile_hook = lambda: holder[0]
    sys.modules["antenv.axon_hooks"] = mod
    antenv.axon_hooks = mod
    if "/root/.axon_site" not in sys.path:
        sys.path.insert(0, "/root/.axon_site")
    from trn_agent_boot.trn_boot import _ntff_profile_via_ctypes
    mod.set_axon_ntff_profile_hook(_ntff_profile_via_ctypes("/opt/axon/libaxon_pjrt.so"))


def kernel(x, attention_mask, wq, wk, wv, wproj, bproj, _trace=False):
    x = np.asarray(x); attention_mask = np.asarray(attention_mask)
    wq = np.asarray(wq); wk = np.asarray(wk); wv = np.asarray(wv)
    wproj = np.asarray(wproj); bproj = np.asarray(bproj)

    if not _is_causal(attention_mask):
        return _numpy_fallback(x, attention_mask, wq, wk, wv, wproj, bproj)

    from concourse import bass_utils
    if _trace:
        _install_ntff_hook()
        bass_utils.upload_artifacts = lambda d: d
    nc = _build_nc()
    in_maps = make_in_maps(x, wq, wk, wv, wproj)
    res = bass_utils.run_bass_kernel_spmd(
        nc, in_maps, core_ids=list(range(NCORES)), trace=_trace)
    out = _assemble(res.results, bproj)
    if _trace:
        return out, res
    return out


# revision 21
# speedup vs baseline: 1.0673x; 1.0673x over previous
"""Multi-head causal attention on 8 Trainium2 NeuronCores.

Problem: B=2, T=2048, C=1024, H=16, HS=64 (fp32), causal mask.

Sharding: 8 cores = 2 batches x 4 head-groups (4 heads each). Each core
computes q/k/v projections + attention + its partial output projection for
its 4 heads of its batch; the host sums the 4 per-batch partials (the
all-reduce of the tensor-parallel output projection) and adds the bias.

Per-core kernel dataflow (everything "transposed", T on the free axis):
  qT/kT [heads(64)x2, T] = W.T @ xT          (PE, K=C chunks of 128)
  v     [T, h, 64+one]                       (PE + strided DVE copy)
  scores: per ts-chunk, BOTH heads of a pair as two K=64 row-tiled
    matmuls (rows 0-63 / 64-127 run concurrently in the PE array);
    causal chunks narrowed to the valid tq range   -> exp (ACT)
  causal: affine_select only on the 128x128 diagonal sub-block (GPSIMD)
  attnT_aug [65, tq] += v_aug.T @ expT       (PE, ones column => row 64 = denom)
  recip = 1/denom straight from PSUM row 64 (DVE), broadcast over 64
    partitions via a K=1 matmul (PE), attnT = attn * recip (DVE)
  y_partial [tq, C] = attnT_pair.T @ wproj   (PE, K=128 per head-pair)

float32r = full-precision fp32 matmul at 1 cycle/row (vs 4 for plain fp32).
"""

import numpy as np

B, T, C, H, HS = 2, 2048, 1024, 16, 64
NCORES = 8
HPC = 4            # heads per core
NKC = C // 128     # contraction chunks (8)
NJ = T // 512      # tq chunks (4)
NTS = T // 128     # ts chunks (16)

_NC_CACHE = {}


def _build_nc():
    if "nc" in _NC_CACHE:
        return _NC_CACHE["nc"]
    from contextlib import ExitStack
    import concourse.bass as bass
    from concourse import bacc, tile, mybir

    f32 = mybir.dt.float32
    f32r = mybir.dt.float32r
    EXP = mybir.ActivationFunctionType.Exp

    nc = bacc.Bacc("TRN2", target_bir_lowering=False, debug=False,
                   enable_asserts=False, num_devices=NCORES)

    xT_d = nc.dram_tensor("xT", (C, T), f32, kind="ExternalInput").ap()
    wq_d = nc.dram_tensor("wq_s", (C, HPC * HS), f32, kind="ExternalInput").ap()
    wk_d = nc.dram_tensor("wk_s", (C, HPC * HS), f32, kind="ExternalInput").ap()
    wv_d = nc.dram_tensor("wv_s", (C, HPC * HS), f32, kind="ExternalInput").ap()
    wp_d = nc.dram_tensor("wp_s", (HPC * HS, C), f32, kind="ExternalInput").ap()
    y_d = nc.dram_tensor("y", (T, C), f32, kind="ExternalOutput").ap()

    scale = float(1.0 / np.sqrt(HS))

    with tile.TileContext(nc) as tc, ExitStack() as ctx:
        persist = ctx.enter_context(tc.tile_pool(name="persist", bufs=1))
        work = ctx.enter_context(tc.tile_pool(name="work", bufs=3))
        small = ctx.enter_context(tc.tile_pool(name="small", bufs=2))
        outp = ctx.enter_context(tc.tile_pool(name="outp", bufs=2))
        spap = ctx.enter_context(tc.tile_pool(name="spap", bufs=3))
        psp = ctx.enter_context(tc.tile_pool(name="psp", bufs=2, space="PSUM"))
        psaux = ctx.enter_context(tc.tile_pool(name="psaux", bufs=2, space="PSUM"))
        psatt = ctx.enter_context(tc.tile_pool(name="psatt", bufs=2, space="PSUM"))

        # ---- persistent SBUF tensors (f32r = fast-fp32 PE path) ----
        xt = [persist.tile([128, T], f32r, tag=f"xt{c}", name=f"xt{c}") for c in range(NKC)]
        wq_sb = persist.tile([128, NKC, 256], f32r, tag="wq")
        wk_sb = persist.tile([128, NKC, 256], f32r, tag="wk")
        wv_sb = persist.tile([128, NKC, 256], f32r, tag="wv")
        wp_sb = persist.tile([128, 2, C], f32r, tag="wp")
        qT = [persist.tile([128, T], f32r, tag=f"qT{p}", name=f"qT{p}") for p in range(2)]
        kT = [persist.tile([128, T], f32r, tag=f"kT{p}", name=f"kT{p}") for p in range(2)]
        # v: [ts-chunk(16) x head(4) x (64 vals + ones col)] per 128 ts partitions
        vt = persist.tile([128, NTS, HPC, 65], f32r, tag="vt")
        attnT = [persist.tile([128, T], f32r, tag=f"attnT{p}", name=f"attnT{p}") for p in range(2)]
        ones64 = persist.tile([1, 64], f32r, tag="ones64")
        ones_f32 = persist.tile([128, 64], f32, tag="ones_f32")

        # ---- loads: striped across sync+scalar in consumption order; gpsimd
        # carries only late-needed wp so it can't steal HBM bandwidth from
        # the critical path (wq + x first halves) ----
        for eng, par in ((nc.sync, 0), (nc.scalar, 1)):
            for c in (par, par + 2):
                eng.dma_start(out=wq_sb[:, c, :],
                              in_=wq_d[c * 128:(c + 1) * 128, :].bitcast(f32r))
            eng.dma_start(out=xt[par][:, 0:1024],
                          in_=xT_d[par * 128:(par + 1) * 128, 0:1024].bitcast(f32r))
            for c in (par + 4, par + 6):
                eng.dma_start(out=wq_sb[:, c, :],
                              in_=wq_d[c * 128:(c + 1) * 128, :].bitcast(f32r))
            for c in (par + 2, par + 4, par + 6):
                eng.dma_start(out=xt[c][:, 0:1024],
                              in_=xT_d[c * 128:(c + 1) * 128, 0:1024].bitcast(f32r))
            for c in range(par, NKC, 2):
                eng.dma_start(out=wk_sb[:, c, :],
                              in_=wk_d[c * 128:(c + 1) * 128, :].bitcast(f32r))
            for c in range(par, NKC, 2):
                eng.dma_start(out=wv_sb[:, c, :],
                              in_=wv_d[c * 128:(c + 1) * 128, :].bitcast(f32r))
            for c in range(par, NKC, 2):
                eng.dma_start(out=xt[c][:, 1024:2048],
                              in_=xT_d[c * 128:(c + 1) * 128, 1024:2048].bitcast(f32r))
        nc.gpsimd.dma_start(out=wp_sb, in_=wp_d.rearrange("(k p) n -> p k n", p=128).bitcast(f32r))

        # f32r matmul operands must come from rounding producers (DVE copy),
        # so memset an f32 tile and cast-copy the ones into place
        nc.vector.memset(ones_f32, 1.0)
        nc.vector.tensor_copy(out=ones64, in_=ones_f32[0:1, 0:64])
        nc.vector.tensor_copy(
            out=vt[:, :, :, 64:65],
            in_=ones_f32.rearrange("p (t h o) -> p t h o", t=NTS, h=HPC, o=1))

        # ---- PE warm-up: dummy matmuls on zeros keep the HAM activity
        # monitor hot through the DMA-only head, so real matmuls start at
        # 2.4 GHz instead of paying the 1.2 GHz cold ramp ----
        warm_f = persist.tile([128, 512], f32, tag="warm_f")
        nc.vector.memset(warm_f, 0.0)
        warm_r = persist.tile([128, 512], f32r, tag="warm_r")
        nc.vector.tensor_copy(out=warm_r, in_=warm_f)
        warm_ps = psaux.tile([128, 512], f32, tag="aux", name="warm_ps")
        for _ in range(60):
            nc.tensor.matmul(warm_ps, lhsT=warm_r[:, 0:128], rhs=warm_r,
                             start=True, stop=True)

        # ---------- emission helpers ----------
        filler = []     # queue of closures emitting independent PE work

        def pull(n):
            for _ in range(n):
                if filler:
                    filler.pop(0)()

        def qk_chain_units(pair, dst, w_sb, J, name):
            # split one 8-matmul accumulation chain into 4 filler units
            ps = psaux.tile([128, 512], f32, tag="aux", name=name)

            def unit(c0):
                def f():
                    for c in (c0, c0 + 1):
                        nc.tensor.matmul(
                            ps,
                            lhsT=w_sb[:, c, 128 * pair:128 * pair + 128],
                            rhs=xt[c][:, 512 * J:512 * J + 512],
                            start=(c == 0), stop=(c == NKC - 1))
                    if c0 == NKC - 2:
                        nc.vector.tensor_copy(
                            out=dst[:, 512 * J:512 * J + 512], in_=ps)
                return f
            return [unit(c0) for c0 in range(0, NKC, 2)]

        def qk_chain(pair, dst, w_sb, J, name):
            for u in qk_chain_units(pair, dst, w_sb, J, name):
                u()

        def v_chain(t):
            ps = psaux.tile([128, 512], f32, tag="aux", name=f"v_{t}")
            for c in range(NKC):
                nc.tensor.matmul(
                    ps[:, 0:256],
                    lhsT=xt[c][:, 128 * t:128 * t + 128],
                    rhs=wv_sb[:, c, :],
                    start=(c == 0), stop=(c == NKC - 1))
            nc.vector.tensor_copy(
                out=vt[:, t, :, 0:64],
                in_=ps[:, 0:256].rearrange("p (h d) -> p h d", h=HPC))

        def proj_tile(m, n):
            py_ = psaux.tile([128, 512], f32, tag="aux", name=f"y_{m}_{n}")
            for pair in range(2):
                nc.tensor.matmul(
                    py_,
                    lhsT=attnT[pair][:, 128 * m:128 * m + 128],
                    rhs=wp_sb[:, pair, 512 * n:512 * n + 512],
                    start=(pair == 0), stop=(pair == 1))
            yo = outp.tile([128, 512], f32, tag="yo")
            nc.vector.tensor_copy(out=yo, in_=py_)
            nc.sync.dma_start(
                out=y_d[128 * m:128 * m + 128, 512 * n:512 * n + 512], in_=yo)

        pending_tail = [None]   # previous block's tail, emitted at u==1

        def flush_tail():
            if pending_tail[0] is not None:
                pending_tail[0]()
                pending_tail[0] = None

        def att_block(pair, J, extra=1):
            """Both heads of `pair` for tq chunk J: per ts-chunk, two K=64
            row-tiled score matmuls (concurrent in the PE array), one batched
            exp, diagonal-narrowed AV accumulation with a ones column giving
            the softmax denominator in row 64."""
            nch = 4 * J + 4
            pa = [psatt.tile([65, 512], f32, tag="att", name=f"pa{hh}_{pair}_{J}")
                  for hh in (0, 1)]

            def do_av(et, t, last):
                d = t - 4 * J
                off = 128 * d if d > 0 else 0
                w = 512 - off
                for hh in (0, 1):
                    base = off if hh == 0 else 512
                    nc.tensor.matmul(
                        pa[hh][:, off:512],
                        lhsT=vt[:, t, 2 * pair + hh, :],
                        rhs=et[:, base:base + w],
                        start=(t == 0), stop=last)

            pend = None          # (et, t) AV one step behind scores
            for t in range(nch):
                d = t - 4 * J
                off = 128 * d if d > 0 else 0
                w = 512 - off
                ss = psp.tile([128, 1024], f32, tag="s", name=f"ss_{pair}_{J}_{t}")
                for hh in (0, 1):
                    # hh0 at cols [off,512), hh1 packed right after at [512,512+w)
                    base = off if hh == 0 else 512
                    nc.tensor.matmul(
                        ss[:, base:base + w],
                        lhsT=kT[pair][64 * hh:64 * hh + 64, 128 * t:128 * t + 128],
                        rhs=qT[pair][64 * hh:64 * hh + 64, 512 * J + off:512 * J + 512],
                        start=True, stop=True)
                et = work.tile([128, 1024], f32r, tag="et", bufs=3)
                nc.scalar.activation(out=et[:, off:512 + w], in_=ss[:, off:512 + w],
                                     func=EXP, scale=scale)
                if d >= 0:
                    for hh in (0, 1):
                        base = off if hh == 0 else 512
                        sl = et[:, base:base + 128]
                        # within the diagonal 128x128 sub-block keep f >= p
                        nc.gpsimd.affine_select(
                            out=sl, in_=sl,
                            compare_op=mybir.AluOpType.is_ge,
                            fill=0.0, base=0,
                            pattern=[[1, 128]], channel_multiplier=-1)
                if pend is not None:
                    do_av(*pend, False)
                pend = (et, t)
                if t == 0:
                    flush_tail()
                pull(extra)
            do_av(*pend, True)

            # softmax denominator: copy pa to SBUF (frees the PSUM banks for
            # the next block) right away; the rest of the normalization is
            # deferred into the next block so it never heads any queue
            spa = []
            for hh in (0, 1):
                sp = spap.tile([65, 512], f32, tag="spa", name=f"spa{hh}_{pair}_{J}")
                nc.vector.tensor_copy(out=sp, in_=pa[hh])
                spa.append(sp)

            def tail():
                for hh in (0, 1):
                    # custom-DVE/gpsimd ops read the wrong partition if bases
                    # mismatch: plain-copy the denom row to partition 0 first
                    den = small.tile([1, 512], f32, tag="den", name=f"den{hh}_{pair}_{J}")
                    nc.vector.tensor_copy(out=den, in_=spa[hh][64:65, :])
                    r = small.tile([1, 512], f32, tag="rc", name=f"rc{hh}_{pair}_{J}")
                    nc.vector.reciprocal_approx_fast(out=r, in_=den)
                    bs = small.tile([64, 512], f32, tag="bs", name=f"bs{hh}_{pair}_{J}")
                    nc.gpsimd.partition_broadcast(bs, r)
                    if hh == 0:
                        nc.vector.tensor_mul(
                            attnT[pair][0:64, 512 * J:512 * J + 512],
                            spa[0][0:64, :], bs)
                    else:
                        tmp = small.tile([64, 512], f32r, tag="tmp")
                        nc.vector.tensor_mul(tmp, spa[1][0:64, :], bs)
                        nc.sync.dma_start(
                            out=attnT[pair][64:128, 512 * J:512 * J + 512], in_=tmp)

            pending_tail[0] = tail

        # ---------- phase A: pair0 q/k for the left half, v left chunks ----------
        qk_chain(0, qT[0], wq_sb, 0, "q0_0")
        qk_chain(0, qT[0], wq_sb, 1, "q0_1")
        qk_chain(0, kT[0], wk_sb, 0, "k0_0")
        qk_chain(0, kT[0], wk_sb, 1, "k0_1")
        for t in range(8):
            v_chain(t)

        # ---------- phase B: attention(pair0) ascending J ----------
        # fillers sized to B's act-latency deficit, in data-arrival order;
        # later-needed chains are emitted inline between blocks instead so
        # phase C's big first block isn't starved
        filler.extend(qk_chain_units(0, qT[0], wq_sb, 2, "q0_2"))
        filler.extend(qk_chain_units(0, kT[0], wk_sb, 2, "k0_2"))
        for t in range(8, 12):
            filler.append(lambda t=t: v_chain(t))
        filler.extend(qk_chain_units(0, qT[0], wq_sb, 3, "q0_3"))
        filler.extend(qk_chain_units(0, kT[0], wk_sb, 3, "k0_3"))
        for t in range(12, NTS):
            filler.append(lambda t=t: v_chain(t))
        filler.extend(qk_chain_units(1, kT[1], wk_sb, 0, "k1_0"))
        filler.extend(qk_chain_units(1, kT[1], wk_sb, 1, "k1_1"))
        att_block(0, 0, extra=2)
        att_block(0, 1, extra=1)
        att_block(0, 2, extra=1)
        att_block(0, 3, extra=1)

        # ---------- phase C: attention(pair1) descending J ----------
        # per-block prerequisites run inline (they fill the previous block's
        # activation drain); proj tiles for finished J rows become fillers
        qk_chain(1, kT[1], wk_sb, 2, "k1_2")
        qk_chain(1, kT[1], wk_sb, 3, "k1_3")
        qk_chain(1, qT[1], wq_sb, 3, "q1_3")
        att_block(1, 3)
        filler.extend(
            (lambda m=m, n=n: (lambda: proj_tile(m, n)))()
            for m in range(12, 16) for n in range(2))
        qk_chain(1, qT[1], wq_sb, 2, "q1_2")
        att_block(1, 2)
        filler.extend(
            (lambda m=m, n=n: (lambda: proj_tile(m, n)))()
            for m in range(8, 12) for n in range(2))
        qk_chain(1, qT[1], wq_sb, 1, "q1_1")
        att_block(1, 1)
        filler.extend(
            (lambda m=m, n=n: (lambda: proj_tile(m, n)))()
            for m in range(4, 8) for n in range(2))
        qk_chain(1, qT[1], wq_sb, 0, "q1_0")
        att_block(1, 0)
        flush_tail()
        filler.extend(
            (lambda m=m, n=n: (lambda: proj_tile(m, n)))()
            for m in range(0, 4) for n in range(2))
        pull(len(filler))

    nc.compile()
    _NC_CACHE["nc"] = nc
    return nc


def make_in_maps(x, wq, wk, wv, wproj):
    xTs = [np.ascontiguousarray(x[b].T) for b in range(B)]
    in_maps = []
    for core in range(NCORES):
        b, g = divmod(core, 4)
        hs = slice(4 * g, 4 * g + 4)
        in_maps.append({
            "xT": xTs[b],
            "wq_s": np.ascontiguousarray(wq[hs].transpose(1, 0, 2).reshape(C, HPC * HS)),
            "wk_s": np.ascontiguousarray(wk[hs].transpose(1, 0, 2).reshape(C, HPC * HS)),
            "wv_s": np.ascontiguousarray(wv[hs].transpose(1, 0, 2).reshape(C, HPC * HS)),
            "wp_s": np.ascontiguousarray(wproj[4 * g * HS:(4 * g + 4) * HS, :]),
        })
    return in_maps


def _assemble(results, bproj):
    y = np.zeros((B, T, C), dtype=np.float32)
    for core in range(NCORES):
        y[core // 4] += results[core]["y"]
    y += bproj.astype(np.float32)[None, None, :]
    return y


def _is_causal(attention_mask):
    tril = np.tril(np.ones((T, T), dtype=bool))
    return all(np.array_equal(attention_mask[b], tril) for b in range(B))


def _numpy_fallback(x, attention_mask, wq, wk, wv, wproj, bproj):
    x64 = x.astype(np.float32)
    q = np.einsum('btc,hcd->bhtd', x64, wq)
    k = np.einsum('btc,hcd->bhtd', x64, wk)
    v = np.einsum('btc,hcd->bhtd', x64, wv)
    wei = np.einsum('bhtd,bhsd->bhts', q, k) / np.sqrt(np.float32(HS))
    wei = np.where(attention_mask[:, None, :, :], wei, -np.inf)
    wei = wei - wei.max(axis=-1, keepdims=True)
    wei = np.exp(wei)
    wei = wei / wei.sum(axis=-1, keepdims=True)
    out = np.einsum('bhts,bhsd->bhtd', wei, v)
    out = out.transpose(0, 2, 1, 3).reshape(B, T, H * HS)
    return (out @ wproj + bproj).astype(np.float32)


def _install_ntff_hook():
    """Recreate the antenv.axon_hooks shim so trace=True works under axon."""
    import sys, types
    try:
        from antenv.axon_hooks import get_axon_ntff_profile_hook  # noqa
        return
    except ImportError:
        pass
    import antenv
    mod = types.ModuleType("antenv.axon_hooks")
    holder = [None]
    mod.set_axon_ntff_profile_hook = lambda h: holder.__setitem__(0, h)
    mod.get_axon_ntff_profile_hook = lambda: holder[0]
    sys.modules["antenv.axon_hooks"] = mod
    antenv.axon_hooks = mod
    if "/root/.axon_site" not in sys.path:
        sys.path.insert(0, "/root/.axon_site")
    from trn_agent_boot.trn_boot import _ntff_profile_via_ctypes
    mod.set_axon_ntff_profile_hook(_ntff_profile_via_ctypes("/opt/axon/libaxon_pjrt.so"))


def kernel(x, attention_mask, wq, wk, wv, wproj, bproj, _trace=False):
    x = np.asarray(x); attention_mask = np.asarray(attention_mask)
    wq = np.asarray(wq); wk = np.asarray(wk); wv = np.asarray(wv)
    wproj = np.asarray(wproj); bproj = np.asarray(bproj)

    if not _is_causal(attention_mask):
        return _numpy_fallback(x, attention_mask, wq, wk, wv, wproj, bproj)

    from concourse import bass_utils
    if _trace:
        _install_ntff_hook()
        bass_utils.upload_artifacts = lambda d: d
    nc = _build_nc()
    in_maps = make_in_maps(x, wq, wk, wv, wproj)
    res = bass_utils.run_bass_kernel_spmd(
        nc, in_maps, core_ids=list(range(NCORES)), trace=_trace)
    out = _assemble(res.results, bproj)
    if _trace:
        return out, res
    return out
